# revision 1
# baseline (speedup 1.0000x reference)
"""Fused masked-softmax attention (DotProductAttention) for 8 TRN2 NeuronCores.

Problem: B=16 batches of Q[2048,64] @ K[2048,64]^T -> mask cols >= valid_len
to -1e6 -> softmax -> @ V[2048,64].

Work decomposition: each batch splits into 4 q-quarters of 512 rows (one
PSUM-bank-wide q-tile each) -> 64 independent units.  Units are sorted by
valid k-tile count nv = ceil(valid_len/128) and dealt into 8 SPMD slots of
8 units (one per core); the compiled program runs slot s with a static
nv_s = max over that slot's units.  K-tiles wholly past a unit's valid_len
contribute exactly 0 (the mask row drives exp to underflow), so the extra
tiles cores run inside a slot are harmless and skipped tiles are exact.
For uniform-random valid_lens this cuts total work to ~50-60% of dense;
worst case (all full) equals the dense kernel.

Per-unit kernel (all on-chip, scores never touch HBM):
  * Layout: S^T[k, q] so softmax's k-reduction becomes a matmul and the
    attn @ V contraction needs no transpose of the big matrix.
  * mm1:  S^T chunk [128k, 512q] = kTa[:, ktile].T @ qTa with AUGMENTED
    bf16 operands: kTa = [K^T; mask_row] (65 rows), qTa = [Q^T; ones].
    The 65th contraction row adds -8e6 to every masked column, so masking
    costs zero instructions.  bf16 streams 1 row/cycle on the PE (fp32 is
    4x, fp32r 2x).
  * exp:  ACT engine (the bottleneck, 1 elem/lane/cycle @1.2GHz),
    exp(0.125 * x) straight out of PSUM in merged N<=1536 activations
    (per-instruction overhead ~350 cycles), bf16 out.
  * mm2:  O^T_aug [65, 512q] = sum_k Vaug[ktile].T @ expS^T[ktile] with
    Vaug = [V | ones] (bf16) -> row 64 accumulates the softmax denominator
    in fp32 PSUM.  Interleaved group-by-group with mm1/exp.
  * finish: copy PSUM->SBUF (f32), PE-transpose 128-col chunks back to
    [q, d] layout, reciprocal of denominator column, per-partition scale,
    one merged DMA per unit.
"""

import functools

import numpy as np
import ml_dtypes

import concourse.bacc as bacc
import concourse.tile as tile
from concourse import mybir
from concourse import bass_utils
from concourse.masks import make_identity

B, LQ, LKV, D = 16, 2048, 2048, 64
N_CORES = 8
KT = 128            # k-tile (partition dim of S^T)
QT = 512            # q-rows per unit (= PSUM bank free dim)
NKT = LKV // KT     # 16
NSLOT = (B * LQ) // (N_CORES * QT)  # 8 units per core
GROUP = 3           # max k-tiles per PSUM tile / merged activation
MASK_RAW = -8.0e6   # * 0.125 scale == -1e6 (reference MASK_VALUE)
F32 = mybir.dt.float32
BF16 = mybir.dt.bfloat16


def _widths(nv):
    """Split nv k-tiles into activation groups of width <=3, avoiding 1-wide
    groups (measured regression) where possible.  2-wide groups go FIRST so
    each slot's first activation has the shortest possible mm1 prefix
    (shrinks the ACT stall at slot boundaries)."""
    threes, rem = divmod(nv, 3)
    if rem == 0:
        return [3] * threes
    if rem == 2:
        return [2] + [3] * threes
    if threes >= 1:
        return [2, 2] + [3] * (threes - 1)
    return [1]


@functools.lru_cache(maxsize=4)
def _build_module(nv_slots):
    nc = bacc.Bacc(None)
    qta_d = nc.dram_tensor("qta", [NSLOT, D + 1, QT], BF16, kind="ExternalInput")
    kta_d = nc.dram_tensor("kta", [NSLOT, D + 1, LKV], BF16, kind="ExternalInput")
    vau_d = nc.dram_tensor("vaug", [128, NSLOT * NKT * (D + 1)], BF16, kind="ExternalInput")
    out_d = nc.dram_tensor("o", [NSLOT, QT, D], F32, kind="ExternalOutput")

    slot_groups = []
    for nv in nv_slots:
        groups, g = [], 0
        for w in _widths(nv):
            groups.append((g, w))
            g += w
        assert g == nv
        slot_groups.append(groups)

    with tile.TileContext(nc) as tc:
        with (
            tc.tile_pool(name="weights", bufs=1) as wpool,
            tc.tile_pool(name="exps", bufs=3) as epool,
            tc.tile_pool(name="ot", bufs=2) as otpool,
            tc.tile_pool(name="recip", bufs=2) as rpool,
            tc.tile_pool(name="outs", bufs=2) as opool,
            tc.tile_pool(name="ps_s", bufs=2, space="PSUM") as ps_s,
            tc.tile_pool(name="ps_o", bufs=1, space="PSUM") as ps_o,
            tc.tile_pool(name="ps_t", bufs=1, space="PSUM") as ps_t,
        ):
            ident = wpool.tile([128, 128], F32, tag="ident")
            make_identity(nc, ident)

            # PE warm-up: dummy matmuls on the identity while the first input
            # DMA is in flight, so the first real mm1s run at full clock
            # (HAM ramps only after sustained PE activity).
            warm = ps_t.tile([128, 128], F32, tag="pt", name="warm")
            for _ in range(6):
                nc.tensor.matmul(warm, lhsT=ident, rhs=ident, start=True, stop=True)

            # Input loads (valid prefix only), in consumption order; the two
            # DMAs the first matmul group needs go out on BOTH HWDGE rings
            # (SP + ACT) in parallel to shorten the critical head path.
            kta_s = [
                wpool.tile(
                    [D + 1, nv_slots[s] * KT], BF16, tag=f"kta{s}", name=f"kta{s}"
                )
                for s in range(NSLOT)
            ]
            qta_s = [
                wpool.tile([D + 1, QT], BF16, tag=f"qta{s}", name=f"qta{s}")
                for s in range(NSLOT)
            ]
            vaug_s = [
                wpool.tile(
                    [128, nv_slots[s] * (D + 1)], BF16, tag=f"vaug{s}", name=f"vaug{s}"
                )
                for s in range(NSLOT)
            ]
            # Process slots big/small interleaved: small slots are dominated
            # by their finish chains (po/pt are single-buffered), which then
            # hide under the neighboring big slots' long ACT windows instead
            # of piling up serially at the kernel tail.  End on the smallest
            # slot so the exposed final compute+finish chain is minimal.
            proc_order = [0, 5, 1, 6, 2, 4, 3, 7][:NSLOT]

            c0 = slot_groups[0][0][1] * KT
            nc.sync.dma_start(out=kta_s[0][:, :c0], in_=kta_d[0, :, :c0])
            nc.scalar.dma_start(out=qta_s[0], in_=qta_d[0])
            nc.sync.dma_start(
                out=kta_s[0][:, c0:], in_=kta_d[0, :, c0 : nv_slots[0] * KT]
            )
            nc.sync.dma_start(out=vaug_s[0], in_=vau_d[:, : nv_slots[0] * (D + 1)])
            for s in proc_order[1:]:
                nc.sync.dma_start(out=qta_s[s], in_=qta_d[s])
                nc.sync.dma_start(out=kta_s[s], in_=kta_d[s, :, : nv_slots[s] * KT])
                nc.sync.dma_start(
                    out=vaug_s[s],
                    in_=vau_d[:, s * NKT * (D + 1) : (s * NKT + nv_slots[s]) * (D + 1)],
                )

            def finish(s, po):
                """Normalize po [65, 512] and store as out[s]."""
                ot = otpool.tile([D + 1, QT], F32, tag="ot", name="ot")
                nc.vector.tensor_copy(ot, po)
                pt = ps_t.tile([128, QT // 128, D + 1], F32, tag="pt", name="pt")
                for j in range(QT // 128):
                    nc.tensor.transpose(
                        pt[:, j, :],
                        ot[:, j * 128 : (j + 1) * 128],
                        ident[: D + 1, : D + 1],
                    )
                rc = rpool.tile([128, QT // 128], F32, tag="rc", name="rc")
                nc.vector.reciprocal(rc, pt[:, :, D])
                ob = opool.tile([128, QT // 128, D], F32, tag="ob", name="ob")
                for j in range(QT // 128):
                    nc.vector.tensor_scalar_mul(
                        ob[:, j, :], pt[:, j, :D], rc[:, j : j + 1]
                    )
                out_ap = out_d[s].rearrange("(j p) d -> p j d", p=128)
                nc.sync.dma_start(out=out_ap, in_=ob)

            for s in proc_order:
                nv = nv_slots[s]
                exps = epool.tile([128, nv * QT], BF16, tag="exps", name="exps")
                po = ps_o.tile([D + 1, QT], F32, tag="po", name="po")
                for g, w in slot_groups[s]:
                    st = ps_s.tile([128, GROUP * QT], F32, tag="st", name="st")
                    for j in range(w):
                        n = g + j
                        nc.tensor.matmul(
                            st[:, j * QT : (j + 1) * QT],
                            lhsT=kta_s[s][:, n * KT : (n + 1) * KT],
                            rhs=qta_s[s],
                            start=True,
                            stop=True,
                        )
                    nc.scalar.activation(
                        out=exps[:, g * QT : (g + w) * QT],
                        in_=st[:, : w * QT],
                        func=mybir.ActivationFunctionType.Exp,
                        scale=0.125,
                    )
                    for j in range(w):
                        n = g + j
                        nc.tensor.matmul(
                            po,
                            lhsT=vaug_s[s][:, n * (D + 1) : (n + 1) * (D + 1)],
                            rhs=exps[:, n * QT : (n + 1) * QT],
                            start=(n == 0),
                            stop=(n == nv - 1),
                            skip_group_check=True,
                        )
                finish(s, po)

    nc.compile()
    return nc


def _plan(valid_lens):
    """Sort the 64 (batch, q-quarter) units by valid k-tile count and deal
    them into NSLOT slots of one unit per core.  Returns (core_units,
    nv_slots) where core_units[c][s] = (batch, quarter)."""
    VL = np.asarray(valid_lens).astype(np.int64)
    nv = np.maximum(1, np.minimum(NKT, (VL + KT - 1) // KT))
    qpb = LQ // QT  # quarters per batch
    unit_nv = np.repeat(nv, qpb)
    order = np.argsort(-unit_nv, kind="stable")
    core_units = [
        [(int(order[NSLOT * s + c]) // qpb, int(order[NSLOT * s + c]) % qpb) for s in range(NSLOT)]
        for c in range(N_CORES)
    ]
    nv_slots = tuple(int(unit_nv[order[NSLOT * s]]) for s in range(NSLOT))
    return core_units, nv_slots


def _shard_inputs(queries, keys, values, valid_lens, core_units):
    """Host-side layout per core: stacked per-unit augmented operands."""
    Q = np.asarray(queries, dtype=np.float32)
    K = np.asarray(keys, dtype=np.float32)
    V = np.asarray(values, dtype=np.float32)
    VL = np.asarray(valid_lens).astype(np.int64)

    cols = np.arange(LKV, dtype=np.int64)
    ones_row = np.ones((1, QT), np.float32)
    in_maps = []
    for c in range(N_CORES):
        qta = np.empty((NSLOT, D + 1, QT), np.float32)
        kta = np.empty((NSLOT, D + 1, LKV), np.float32)
        va = np.empty((128, NSLOT * NKT * (D + 1)), np.float32)
        for s, (b, qt) in enumerate(core_units[c]):
            qta[s] = np.concatenate(
                [Q[b, qt * QT : (qt + 1) * QT, :].T, ones_row], axis=0
            )
            mask = np.where(cols >= VL[b], MASK_RAW, 0.0).astype(np.float32)
            kta[s] = np.concatenate([K[b].T, mask[None, :]], axis=0)
            vb = np.concatenate([V[b], np.ones((LKV, 1), np.float32)], axis=-1)
            va[:, s * NKT * (D + 1) : (s + 1) * NKT * (D + 1)] = (
                vb.reshape(NKT, KT, D + 1).transpose(1, 0, 2).reshape(128, -1)
            )
        in_maps.append(
            {
                "qta": qta.astype(ml_dtypes.bfloat16),
                "kta": kta.astype(ml_dtypes.bfloat16),
                "vaug": va.astype(ml_dtypes.bfloat16),
            }
        )
    return in_maps


def kernel(queries, keys, values, valid_lens):
    core_units, nv_slots = _plan(valid_lens)
    nc = _build_module(nv_slots)
    in_maps = _shard_inputs(queries, keys, values, valid_lens, core_units)
    res = bass_utils.run_bass_kernel_spmd(nc, in_maps, core_ids=list(range(N_CORES)))
    out = np.empty((B, LQ, D), np.float32)
    for c in range(N_CORES):
        o = res.results[c]["o"].reshape(NSLOT, QT, D)
        for s, (b, qt) in enumerate(core_units[c]):
            out[b, qt * QT : (qt + 1) * QT, :] = o[s]
    return out



# revision 3
# speedup vs baseline: 1.0242x; 1.0242x over previous
"""Fused masked-softmax attention (DotProductAttention) for 8 TRN2 NeuronCores.

Problem: B=16 batches of Q[2048,64] @ K[2048,64]^T -> mask cols >= valid_len
to -1e6 -> softmax -> @ V[2048,64].

Work decomposition: each batch splits into 4 q-quarters of 512 rows (one
PSUM-bank-wide q-tile each) -> 64 independent units.  Units are sorted by
valid k-tile count nv = ceil(valid_len/128) and dealt into 8 SPMD slots of
8 units (one per core); the compiled program runs slot s with a static
nv_s = max over that slot's units.  The mask lives entirely in the V-side
operand (see below), so k-tiles past a unit's valid_len contribute exactly
0 and skipped tiles are exact.  For uniform-random valid_lens this cuts
total work to ~50-60% of dense; worst case (all full) equals dense.

Per-unit kernel (all on-chip, scores never touch HBM):
  * Layout: S^T[k, q] so softmax's k-reduction becomes a matmul and the
    attn @ V contraction needs no transpose of the big matrix.
  * MASKING: entirely in Vaug = [V | ones] with rows k >= valid_len zeroed
    on the host.  Invalid keys then add exactly 0 to both the numerator
    (V side) and the denominator (ones side) -- identical to exp(-1e6)=0
    in the reference.  Scores on invalid keys are real (bounded ~e^5 after
    the 0.125 scale), never masked, and simply multiplied by zero.
  * mm1 contraction is therefore pure K^T.T @ Q^T = 64 rows -> ROW-TILED
    PAIRS: tile 2j loads weights into PE array rows 0-63, tile 2j+1 into
    rows 64-127 (kta is host-packed [128, npairs*128]; Q^T is duplicated
    into SBUF partitions 64-127 so rhs/lhsT base partitions match).  The
    two matmuls of a pair run CONCURRENTLY in the array -> mm1 streams
    ~2 k-tiles per 512 cycles, half the v1 cost.
  * exp: ACT engine (the bottleneck: 1 elem/lane/cycle @ 1.2GHz),
    exp(0.125 * x) straight out of PSUM in merged N<=1536 activations
    (per-instruction overhead ~350 cycles), bf16 out.  The exp table is
    preloaded at t~4.5us by a dummy 1-element activation so the ~2.7us
    ACT_TABLE_LOAD overlaps the input DMAs.
  * mm2:  O^T_aug [65, 512q] = sum_k Vaug[ktile].T @ expS^T[ktile] in
    fp32 PSUM; row 64 accumulates the softmax denominator.
  * Pipelining: matmul groups are emitted with a one-group global
    lookahead (mm1[i+1] before exp[i]/mm2[i]) so mm1 pairs stay adjacent
    in the PE queue across group and slot boundaries and ACT never waits
    at slot boundaries.
  * Head: all input DMAs are issued first on the sync/gpsimd queues
    (Scalar queue stays free for ACT), and a short bf16 warmup matmul
    chain starts the HAM clock-ungate (~3.4us of sustained PE activity)
    during the input-DMA wait.
  * finish: copy PSUM->SBUF (f32), PE-transpose 128-col chunks back to
    [q, d] layout, reciprocal of denominator column, per-partition scale,
    one merged DMA per unit issued from the vector queue.
"""

import functools

import numpy as np
import ml_dtypes

import concourse.bacc as bacc
import concourse.tile as tile
from concourse import mybir
from concourse import bass_utils
from concourse.masks import make_identity

B, LQ, LKV, D = 16, 2048, 2048, 64
N_CORES = 8
KT = 128            # k-tile (partition dim of S^T)
QT = 512            # q-rows per unit (= PSUM bank free dim)
NKT = LKV // KT     # 16
NPAIR = NKT // 2    # 8 pair blocks in kta
NSLOT = (B * LQ) // (N_CORES * QT)  # 8 units per core
GROUP = 3           # max k-tiles per PSUM tile / merged activation
F32 = mybir.dt.float32
BF16 = mybir.dt.bfloat16


def _widths(nv):
    """Split nv k-tiles into activation groups of width <=3, avoiding 1-wide
    groups (measured regression) where possible.  2-wide groups go FIRST so
    each slot's first activation has the shortest possible mm1 prefix."""
    threes, rem = divmod(nv, 3)
    if rem == 0:
        return [3] * threes
    if rem == 2:
        return [2] + [3] * threes
    if threes >= 1:
        return [2, 2] + [3] * (threes - 1)
    return [1]


@functools.lru_cache(maxsize=4)
def _build_module(nv_slots):
    nc = bacc.Bacc(None)
    qta_d = nc.dram_tensor("qta", [128, NSLOT * QT], BF16, kind="ExternalInput")
    kta_d = nc.dram_tensor("kta", [NSLOT, 128, NPAIR * KT], BF16, kind="ExternalInput")
    vau_d = nc.dram_tensor("vaug", [128, NSLOT * NKT * (D + 1)], BF16, kind="ExternalInput")
    out_d = nc.dram_tensor("o", [NSLOT, QT, D], F32, kind="ExternalOutput")

    slot_groups = []
    for nv in nv_slots:
        groups, g = [], 0
        for w in _widths(nv):
            groups.append((g, w))
            g += w
        assert g == nv
        slot_groups.append(groups)

    # Process slots big/small interleaved: small slots are dominated by their
    # finish chains, which hide under the neighboring big slots' long ACT
    # windows.  End on the smallest slot so the exposed tail is minimal.
    proc_order = [0, 5, 1, 6, 2, 4, 3, 7][:NSLOT]
    # flattened (slot, group_start, width) sequence in emission order
    seq = [(s, g, w) for s in proc_order for (g, w) in slot_groups[s]]

    with tile.TileContext(nc) as tc:
        with (
            tc.tile_pool(name="weights", bufs=1) as wpool,
            tc.tile_pool(name="exps", bufs=3) as epool,
            tc.tile_pool(name="ot", bufs=2) as otpool,
            tc.tile_pool(name="recip", bufs=2) as rpool,
            tc.tile_pool(name="outs", bufs=2) as opool,
            tc.tile_pool(name="ps_s", bufs=2, space="PSUM") as ps_s,
            tc.tile_pool(name="ps_o", bufs=1, space="PSUM") as ps_o,
            tc.tile_pool(name="ps_t", bufs=1, space="PSUM") as ps_t,
        ):
            # --- tiny setup tiles on gpsimd so dependents start early
            adum = wpool.tile([128, 1], F32, tag="adum", name="adum")
            nc.gpsimd.memset(adum, 0.0)
            wk = wpool.tile([64, KT], BF16, tag="wk", name="wk")
            nc.gpsimd.memset(wk, 0.125)
            wq = wpool.tile([64, QT], BF16, tag="wq", name="wq")
            nc.gpsimd.memset(wq, 0.125)

            # --- input SBUF tiles
            qta_all = wpool.tile([128, NSLOT * QT], BF16, tag="qta", name="qta")
            kta_s = [
                wpool.tile(
                    [128, ((nv_slots[s] + 1) // 2) * KT], BF16,
                    tag=f"kta{s}", name=f"kta{s}",
                )
                for s in range(NSLOT)
            ]
            vaug_s = [
                wpool.tile(
                    [128, nv_slots[s] * (D + 1)], BF16,
                    tag=f"vaug{s}", name=f"vaug{s}",
                )
                for s in range(NSLOT)
            ]

            s0 = proc_order[0]
            np0 = (nv_slots[s0] + 1) // 2
            c0 = ((slot_groups[s0][0][1] + 1) // 2) * KT  # first group's pair block(s)
            # sync queue: slot-0 critical inputs first, then the rest
            nc.sync.dma_start(out=qta_all[:, s0 * QT : (s0 + 1) * QT],
                              in_=qta_d[:, s0 * QT : (s0 + 1) * QT])
            nc.sync.dma_start(out=kta_s[s0][:, :c0], in_=kta_d[s0, :, :c0])

            # scalar queue: preload the exp ACT table during the DMA wait
            aout = wpool.tile([128, 1], F32, tag="aout", name="aout")
            nc.scalar.activation(out=aout, in_=adum,
                                 func=mybir.ActivationFunctionType.Exp)

            # gpsimd queue: vaug for slot 0, identity, then remaining vaug
            nc.gpsimd.dma_start(
                out=vaug_s[s0],
                in_=vau_d[:, s0 * NKT * (D + 1) : (s0 * NKT + nv_slots[s0]) * (D + 1)],
            )
            ident = wpool.tile([128, 128], F32, tag="ident")
            make_identity(nc, ident)

            # tensor queue: bf16 warmup chain to start the HAM ungate early
            warm = ps_s.tile([128, GROUP * QT], F32, tag="st", name="warm")
            for _ in range(5):
                nc.tensor.matmul(warm[:, :QT], lhsT=wk, rhs=wq, start=True, stop=True)

            # rest of the inputs, in consumption order
            rest = [s for s in proc_order if s != s0]
            nc.sync.dma_start(
                out=kta_s[s0][:, c0:], in_=kta_d[s0, :, c0 : np0 * KT]
            )
            for s in rest:
                nc.sync.dma_start(out=qta_all[:, s * QT : (s + 1) * QT],
                                  in_=qta_d[:, s * QT : (s + 1) * QT])
                nc.sync.dma_start(
                    out=kta_s[s], in_=kta_d[s, :, : ((nv_slots[s] + 1) // 2) * KT]
                )
            for s in rest:
                nc.gpsimd.dma_start(
                    out=vaug_s[s],
                    in_=vau_d[:, s * NKT * (D + 1) : (s * NKT + nv_slots[s]) * (D + 1)],
                )

            def finish(s, po):
                """Normalize po [65, 512] and store as out[s]."""
                ot = otpool.tile([D + 1, QT], F32, tag="ot", name="ot")
                nc.vector.tensor_copy(ot, po)
                pt = ps_t.tile([128, QT // 128, D + 1], F32, tag="pt", name="pt")
                for j in range(QT // 128):
                    nc.tensor.transpose(
                        pt[:, j, :],
                        ot[:, j * 128 : (j + 1) * 128],
                        ident[: D + 1, : D + 1],
                    )
                rc = rpool.tile([128, QT // 128], F32, tag="rc", name="rc")
                nc.vector.reciprocal(rc, pt[:, :, D])
                ob = opool.tile([128, QT // 128, D], F32, tag="ob", name="ob")
                for j in range(QT // 128):
                    nc.vector.tensor_scalar_mul(
                        ob[:, j, :], pt[:, j, :D], rc[:, j : j + 1]
                    )
                out_ap = out_d[s].rearrange("(j p) d -> p j d", p=128)
                nc.sync.dma_start(out=out_ap, in_=ob)

            exps_t, po_t, st_t = {}, {}, {}

            def emit_m1(i):
                s, g, w = seq[i]
                if s not in exps_t:
                    exps_t[s] = epool.tile(
                        [128, nv_slots[s] * QT], BF16, tag="exps", name=f"exps{s}"
                    )
                    po_t[s] = ps_o.tile([D + 1, QT], F32, tag="po", name=f"po{s}")
                st = ps_s.tile([128, GROUP * QT], F32, tag="st", name=f"st{i}")
                st_t[i] = st
                for j in range(w):
                    n = g + j
                    h, jp = n % 2, n // 2
                    nc.tensor.matmul(
                        st[:, j * QT : (j + 1) * QT],
                        lhsT=kta_s[s][h * 64 : (h + 1) * 64, jp * KT : (jp + 1) * KT],
                        rhs=qta_all[h * 64 : (h + 1) * 64, s * QT : (s + 1) * QT],
                        start=True,
                        stop=True,
                    )

            emit_m1(0)
            for i, (s, g, w) in enumerate(seq):
                if i + 1 < len(seq):
                    emit_m1(i + 1)
                st = st_t.pop(i)
                nv = nv_slots[s]
                nc.scalar.activation(
                    out=exps_t[s][:, g * QT : (g + w) * QT],
                    in_=st[:, : w * QT],
                    func=mybir.ActivationFunctionType.Exp,
                    scale=0.125,
                )
                for j in range(w):
                    n = g + j
                    nc.tensor.matmul(
                        po_t[s],
                        lhsT=vaug_s[s][:, n * (D + 1) : (n + 1) * (D + 1)],
                        rhs=exps_t[s][:, n * QT : (n + 1) * QT],
                        start=(n == 0),
                        stop=(n == nv - 1),
                        skip_group_check=True,
                    )
                if g + w == nv:
                    finish(s, po_t[s])

    nc.compile()
    return nc


def _plan(valid_lens):
    """Sort the 64 (batch, q-quarter) units by valid k-tile count and deal
    them into NSLOT slots of one unit per core.  Returns (core_units,
    nv_slots) where core_units[c][s] = (batch, quarter)."""
    VL = np.asarray(valid_lens).astype(np.int64)
    nv = np.maximum(1, np.minimum(NKT, (VL + KT - 1) // KT))
    qpb = LQ // QT  # quarters per batch
    unit_nv = np.repeat(nv, qpb)
    order = np.argsort(-unit_nv, kind="stable")
    core_units = [
        [(int(order[NSLOT * s + c]) // qpb, int(order[NSLOT * s + c]) % qpb) for s in range(NSLOT)]
        for c in range(N_CORES)
    ]
    nv_slots = tuple(int(unit_nv[order[NSLOT * s]]) for s in range(NSLOT))
    return core_units, nv_slots


def _shard_inputs(queries, keys, values, valid_lens, core_units):
    """Host-side layout per core: stacked per-unit operands.

    kta: pure K^T pair-packed [128, NPAIR*128]: partition (h*64+d), col
         (j*128+c) = K[b][(2j+h)*128+c, d] -- tile 2j in partitions 0-63,
         tile 2j+1 in partitions 64-127 (PE row-tiling halves).
    qta: Q^T quarter duplicated into partitions 64-127: [128, 512].
    vaug: [V | ones] with rows k >= valid_len zeroed (the mask lives here).
    """
    Q = np.asarray(queries, dtype=np.float32)
    K = np.asarray(keys, dtype=np.float32)
    V = np.asarray(values, dtype=np.float32)
    VL = np.asarray(valid_lens).astype(np.int64)

    cols = np.arange(LKV, dtype=np.int64)
    in_maps = []
    for c in range(N_CORES):
        qta = np.empty((128, NSLOT * QT), np.float32)
        kta = np.empty((NSLOT, 128, NPAIR * KT), np.float32)
        va = np.empty((128, NSLOT * NKT * (D + 1)), np.float32)
        for s, (b, qt) in enumerate(core_units[c]):
            qT = Q[b, qt * QT : (qt + 1) * QT, :].T  # [64, 512]
            qta[:64, s * QT : (s + 1) * QT] = qT
            qta[64:, s * QT : (s + 1) * QT] = qT
            # K[b] [2048, 64] -> [NPAIR, 2, KT, D] -> (h, d, j, c)
            kta[s] = (
                K[b].reshape(NPAIR, 2, KT, D).transpose(1, 3, 0, 2).reshape(128, -1)
            )
            vb = np.concatenate([V[b], np.ones((LKV, 1), np.float32)], axis=-1)
            vb *= (cols < VL[b])[:, None]
            va[:, s * NKT * (D + 1) : (s + 1) * NKT * (D + 1)] = (
                vb.reshape(NKT, KT, D + 1).transpose(1, 0, 2).reshape(128, -1)
            )
        in_maps.append(
            {
                "qta": qta.astype(ml_dtypes.bfloat16),
                "kta": kta.astype(ml_dtypes.bfloat16),
                "vaug": va.astype(ml_dtypes.bfloat16),
            }
        )
    return in_maps


def kernel(queries, keys, values, valid_lens):
    core_units, nv_slots = _plan(valid_lens)
    nc = _build_module(nv_slots)
    in_maps = _shard_inputs(queries, keys, values, valid_lens, core_units)
    res = bass_utils.run_bass_kernel_spmd(nc, in_maps, core_ids=list(range(N_CORES)))
    out = np.empty((B, LQ, D), np.float32)
    for c in range(N_CORES):
        o = res.results[c]["o"].reshape(NSLOT, QT, D)
        for s, (b, qt) in enumerate(core_units[c]):
            out[b, qt * QT : (qt + 1) * QT, :] = o[s]
    return out


# revision 11
# speedup vs baseline: 1.0665x; 1.0413x over previous
"""Fused masked-softmax attention (DotProductAttention) for 8 TRN2 NeuronCores.

Problem: B=16 batches of Q[2048,64] @ K[2048,64]^T -> mask cols >= valid_len
to -1e6 -> softmax -> @ V[2048,64].

Work decomposition: each batch splits into 4 q-quarters of 512 rows (one
PSUM-bank-wide q-tile each) -> 64 independent units.  Units are sorted by
valid k-tile count nv = ceil(valid_len/128) and dealt into 8 SPMD slots of
8 units (one per core); the compiled program runs slot s with a static
nv_s = max over that slot's units.  The mask lives entirely in the V-side
operand (see below), so k-tiles past a unit's valid_len contribute exactly
0 and skipped tiles are exact.  For uniform-random valid_lens this cuts
total work to ~50-60% of dense; worst case (all full) equals dense.

Per-unit kernel (all on-chip, scores never touch HBM):
  * Layout: S^T[k, q] so softmax's k-reduction becomes a matmul and the
    attn @ V contraction needs no transpose of the big matrix.
  * MASKING: entirely in Vaug = [V | ones] with rows k >= valid_len zeroed
    on the host.  Invalid keys then add exactly 0 to both the numerator
    (V side) and the denominator (ones side) -- identical to exp(-1e6)=0
    in the reference.  Scores on invalid keys are real (bounded ~e^5 after
    the 0.125 scale), never masked, and simply multiplied by zero.
  * mm1 contraction is therefore pure K^T.T @ Q^T = 64 rows -> ROW-TILED
    PAIRS: tile 2j loads weights into PE array rows 0-63, tile 2j+1 into
    rows 64-127 (kta is host-packed [128, npairs*128]; Q^T is duplicated
    into SBUF partitions 64-127 so rhs/lhsT base partitions match).  The
    two matmuls of a pair run CONCURRENTLY in the array -> mm1 streams
    ~2 k-tiles per 512 cycles, half the v1 cost.
  * exp: ACT engine (the bottleneck: 1 elem/lane/cycle @ 1.2GHz),
    exp(0.125 * x) straight out of PSUM in merged N<=1536 activations
    (per-instruction overhead ~350 cycles), bf16 out.  The exp table is
    preloaded at t~4.5us by a dummy 1-element activation so the ~2.7us
    ACT_TABLE_LOAD overlaps the input DMAs.
  * mm2:  O^T_aug [65, 512q] = sum_k Vaug[ktile].T @ expS^T[ktile] in
    fp32 PSUM; row 64 accumulates the softmax denominator.
  * Pipelining: matmul groups are emitted with a one-group global
    lookahead (mm1[i+1] before exp[i]/mm2[i]) so mm1 pairs stay adjacent
    in the PE queue across group and slot boundaries and ACT never waits
    at slot boundaries.
  * Head: all input DMAs are issued first on the sync/gpsimd queues
    (Scalar queue stays free for ACT), and a short bf16 warmup matmul
    chain starts the HAM clock-ungate (~3.4us of sustained PE activity)
    during the input-DMA wait.
  * finish: copy PSUM->SBUF (f32), PE-transpose 128-col chunks back to
    [q, d] layout, reciprocal of denominator column, per-partition scale,
    one merged DMA per unit issued from the vector queue.
"""

import functools

import numpy as np
import ml_dtypes

import concourse.bacc as bacc
import concourse.tile as tile
from concourse import mybir
from concourse import bass_utils
from concourse.masks import make_identity

B, LQ, LKV, D = 16, 2048, 2048, 64
N_CORES = 8
KT = 128            # k-tile (partition dim of S^T)
QT = 512            # q-rows per unit (= PSUM bank free dim)
NKT = LKV // KT     # 16
NPAIR = NKT // 2    # 8 pair blocks in kta
NSLOT = (B * LQ) // (N_CORES * QT)  # 8 units per core
GROUP = 3           # max k-tiles per PSUM tile / merged activation
PROC_ORDER = [0, 5, 1, 4, 2, 3, 6, 7]  # slot processing order (tail = smallest)
F32 = mybir.dt.float32
BF16 = mybir.dt.bfloat16


def _widths(nv):
    """Split nv k-tiles into activation groups of width <=3, avoiding 1-wide
    groups (measured regression) where possible.  2-wide groups go FIRST so
    each slot's first activation has the shortest possible mm1 prefix."""
    threes, rem = divmod(nv, 3)
    if rem == 0:
        return [3] * threes
    if rem == 2:
        return [2] + [3] * threes
    if threes >= 1:
        return [2, 2] + [3] * (threes - 1)
    return [1]


@functools.lru_cache(maxsize=4)
def _build_module(nv_slots):
    nc = bacc.Bacc(None)
    qta_d = nc.dram_tensor("qta", [128, NSLOT * QT], BF16, kind="ExternalInput")
    kta_d = nc.dram_tensor("kta", [NSLOT, 128, NPAIR * KT], BF16, kind="ExternalInput")
    vau_d = nc.dram_tensor("vaug", [128, NSLOT * NKT * (D + 1)], BF16, kind="ExternalInput")
    out_d = nc.dram_tensor("o", [NSLOT, QT, D], BF16, kind="ExternalOutput")
    # last-processed slot: raw O^T + denominator row, normalized on host
    # (skips the transpose/normalize chain that would serialize at the tail)
    o2_d = nc.dram_tensor("o2", [D + 1, QT], F32, kind="ExternalOutput")

    slot_groups = []
    for nv in nv_slots:
        groups, g = [], 0
        for w in _widths(nv):
            groups.append((g, w))
            g += w
        assert g == nv
        slot_groups.append(groups)

    # Process slots big/small interleaved: small slots are dominated by their
    # finish chains, which hide under the neighboring big slots' long ACT
    # windows.  End on the two smallest slots so the exposed tail is minimal.
    proc_order = PROC_ORDER[:NSLOT]
    # flattened (slot, group_start, width) sequence in emission order
    seq = [(s, g, w) for s in proc_order for (g, w) in slot_groups[s]]

    with tile.TileContext(nc) as tc:
        with (
            tc.tile_pool(name="weights", bufs=1) as wpool,
            tc.tile_pool(name="exps", bufs=3) as epool,
            tc.tile_pool(name="ot", bufs=2) as otpool,
            tc.tile_pool(name="recip", bufs=2) as rpool,
            tc.tile_pool(name="outs", bufs=2) as opool,
            tc.tile_pool(name="ps_s", bufs=2, space="PSUM") as ps_s,
            tc.tile_pool(name="ps_o", bufs=1, space="PSUM") as ps_o,
            tc.tile_pool(name="ps_t", bufs=1, space="PSUM") as ps_t,
        ):
            # --- tiny setup tiles on gpsimd so dependents start early
            adum = wpool.tile([128, 1], F32, tag="adum", name="adum")
            nc.gpsimd.memset(adum, 0.0)
            wk = wpool.tile([64, KT], BF16, tag="wk", name="wk")
            nc.gpsimd.memset(wk, 0.125)
            wq = wpool.tile([64, QT], BF16, tag="wq", name="wq")
            nc.gpsimd.memset(wq, 0.125)

            # --- input SBUF tiles
            qta_all = wpool.tile([128, NSLOT * QT], BF16, tag="qta", name="qta")
            kta_s = [
                wpool.tile(
                    [128, ((nv_slots[s] + 1) // 2) * KT], BF16,
                    tag=f"kta{s}", name=f"kta{s}",
                )
                for s in range(NSLOT)
            ]
            vaug_s = [
                wpool.tile(
                    [128, nv_slots[s] * (D + 1)], BF16,
                    tag=f"vaug{s}", name=f"vaug{s}",
                )
                for s in range(NSLOT)
            ]

            s0 = proc_order[0]
            np0 = (nv_slots[s0] + 1) // 2
            c0 = ((slot_groups[s0][0][1] + 1) // 2) * KT  # first group's pair block(s)
            # sync queue: slot-0 critical inputs first, then the rest
            nc.sync.dma_start(out=qta_all[:, s0 * QT : (s0 + 1) * QT],
                              in_=qta_d[:, s0 * QT : (s0 + 1) * QT])
            nc.sync.dma_start(out=kta_s[s0][:, :c0], in_=kta_d[s0, :, :c0])

            # scalar queue: preload the exp ACT table during the DMA wait
            aout = wpool.tile([128, 1], F32, tag="aout", name="aout")
            nc.scalar.activation(out=aout, in_=adum,
                                 func=mybir.ActivationFunctionType.Exp)

            # gpsimd queue: vaug for slot 0, identity, then remaining vaug
            nc.gpsimd.dma_start(
                out=vaug_s[s0],
                in_=vau_d[:, s0 * NKT * (D + 1) : (s0 * NKT + nv_slots[s0]) * (D + 1)],
            )
            ident = wpool.tile([128, 128], F32, tag="ident")
            make_identity(nc, ident)

            # tensor queue: bf16 warmup chain keeps the PE busy through the
            # input-DMA wait (HAM/power-arbiter activity clock)
            warm = ps_s.tile([128, GROUP * QT], F32, tag="st", name="warm")
            for _ in range(8):
                nc.tensor.matmul(warm[:, :QT], lhsT=wk, rhs=wq, start=True, stop=True)

            # rest of the inputs, in consumption order; kta on sync,
            # vaug+qta interleaved on gpsimd
            rest = [s for s in proc_order if s != s0]
            nc.sync.dma_start(
                out=kta_s[s0][:, c0:], in_=kta_d[s0, :, c0 : np0 * KT]
            )
            for s in rest:
                nc.sync.dma_start(
                    out=kta_s[s], in_=kta_d[s, :, : ((nv_slots[s] + 1) // 2) * KT]
                )
            for s in rest:
                nc.gpsimd.dma_start(
                    out=vaug_s[s],
                    in_=vau_d[:, s * NKT * (D + 1) : (s * NKT + nv_slots[s]) * (D + 1)],
                )
                nc.gpsimd.dma_start(out=qta_all[:, s * QT : (s + 1) * QT],
                                    in_=qta_d[:, s * QT : (s + 1) * QT])

            def finish(s, po):
                """Normalize po [65, 512] and store as out[s]."""
                ot = otpool.tile([D + 1, QT], F32, tag="ot", name="ot")
                nc.vector.tensor_copy(ot, po)
                if s == proc_order[-1]:
                    # last slot: ship raw O^T+denom, normalize on host
                    nc.sync.dma_start(out=o2_d[:, :], in_=ot)
                    return
                pt = ps_t.tile([128, QT // 128, D + 1], F32, tag="pt", name="pt")
                for j in range(QT // 128):
                    nc.tensor.transpose(
                        pt[:, j, :],
                        ot[:, j * 128 : (j + 1) * 128],
                        ident[: D + 1, : D + 1],
                    )
                rc = rpool.tile([128, QT // 128], F32, tag="rc", name="rc")
                nc.vector.reciprocal(rc, pt[:, :, D])
                ob = opool.tile([128, QT // 128, D], BF16, tag="ob", name="ob")
                for j in range(QT // 128):
                    nc.vector.tensor_scalar_mul(
                        ob[:, j, :], pt[:, j, :D], rc[:, j : j + 1]
                    )
                out_ap = out_d[s].rearrange("(j p) d -> p j d", p=128)
                nc.sync.dma_start(out=out_ap, in_=ob)

            exps_t, po_t, st_t = {}, {}, {}

            def emit_m1(i):
                s, g, w = seq[i]
                if s not in exps_t:
                    exps_t[s] = epool.tile(
                        [128, nv_slots[s] * QT], BF16, tag="exps", name=f"exps{s}"
                    )
                    po_t[s] = ps_o.tile([D + 1, QT], F32, tag="po", name=f"po{s}")
                st = ps_s.tile([128, GROUP * QT], F32, tag="st", name=f"st{i}")
                st_t[i] = st
                for j in range(w):
                    n = g + j
                    h, jp = n % 2, n // 2
                    nc.tensor.matmul(
                        st[:, j * QT : (j + 1) * QT],
                        lhsT=kta_s[s][h * 64 : (h + 1) * 64, jp * KT : (jp + 1) * KT],
                        rhs=qta_all[h * 64 : (h + 1) * 64, s * QT : (s + 1) * QT],
                        start=True,
                        stop=True,
                    )

            emit_m1(0)
            for i, (s, g, w) in enumerate(seq):
                if i + 1 < len(seq):
                    emit_m1(i + 1)
                st = st_t.pop(i)
                nv = nv_slots[s]
                nc.scalar.activation(
                    out=exps_t[s][:, g * QT : (g + w) * QT],
                    in_=st[:, : w * QT],
                    func=mybir.ActivationFunctionType.Exp,
                    scale=0.125,
                )
                for j in range(w):
                    n = g + j
                    nc.tensor.matmul(
                        po_t[s],
                        lhsT=vaug_s[s][:, n * (D + 1) : (n + 1) * (D + 1)],
                        rhs=exps_t[s][:, n * QT : (n + 1) * QT],
                        start=(n == 0),
                        stop=(n == nv - 1),
                        skip_group_check=True,
                    )
                if g + w == nv:
                    finish(s, po_t[s])

    nc.compile()
    return nc


def _plan(valid_lens):
    """Sort the 64 (batch, q-quarter) units by valid k-tile count and deal
    them into NSLOT slots of one unit per core.  Returns (core_units,
    nv_slots) where core_units[c][s] = (batch, quarter)."""
    VL = np.asarray(valid_lens).astype(np.int64)
    nv = np.maximum(1, np.minimum(NKT, (VL + KT - 1) // KT))
    qpb = LQ // QT  # quarters per batch
    unit_nv = np.repeat(nv, qpb)
    order = np.argsort(-unit_nv, kind="stable")
    core_units = [
        [(int(order[NSLOT * s + c]) // qpb, int(order[NSLOT * s + c]) % qpb) for s in range(NSLOT)]
        for c in range(N_CORES)
    ]
    nv_slots = tuple(int(unit_nv[order[NSLOT * s]]) for s in range(NSLOT))
    return core_units, nv_slots


def _shard_inputs(queries, keys, values, valid_lens, core_units):
    """Host-side layout per core: stacked per-unit operands.

    kta: pure K^T pair-packed [128, NPAIR*128]: partition (h*64+d), col
         (j*128+c) = K[b][(2j+h)*128+c, d] -- tile 2j in partitions 0-63,
         tile 2j+1 in partitions 64-127 (PE row-tiling halves).
    qta: Q^T quarter duplicated into partitions 64-127: [128, 512].
    vaug: [V | ones] with rows k >= valid_len zeroed (the mask lives here).
    """
    Q = np.asarray(queries, dtype=np.float32)
    K = np.asarray(keys, dtype=np.float32)
    V = np.asarray(values, dtype=np.float32)
    VL = np.asarray(valid_lens).astype(np.int64)

    cols = np.arange(LKV, dtype=np.int64)
    in_maps = []
    for c in range(N_CORES):
        qta = np.empty((128, NSLOT * QT), np.float32)
        kta = np.empty((NSLOT, 128, NPAIR * KT), np.float32)
        va = np.empty((128, NSLOT * NKT * (D + 1)), np.float32)
        for s, (b, qt) in enumerate(core_units[c]):
            qT = Q[b, qt * QT : (qt + 1) * QT, :].T  # [64, 512]
            qta[:64, s * QT : (s + 1) * QT] = qT
            qta[64:, s * QT : (s + 1) * QT] = qT
            # K[b] [2048, 64] -> [NPAIR, 2, KT, D] -> (h, d, j, c)
            kta[s] = (
                K[b].reshape(NPAIR, 2, KT, D).transpose(1, 3, 0, 2).reshape(128, -1)
            )
            vb = np.concatenate([V[b], np.ones((LKV, 1), np.float32)], axis=-1)
            vb *= (cols < VL[b])[:, None]
            va[:, s * NKT * (D + 1) : (s + 1) * NKT * (D + 1)] = (
                vb.reshape(NKT, KT, D + 1).transpose(1, 0, 2).reshape(128, -1)
            )
        in_maps.append(
            {
                "qta": qta.astype(ml_dtypes.bfloat16),
                "kta": kta.astype(ml_dtypes.bfloat16),
                "vaug": va.astype(ml_dtypes.bfloat16),
            }
        )
    return in_maps


def _gather(res, core_units):
    """Assemble the full [B, LQ, D] f32 output from per-core results."""
    last = PROC_ORDER[NSLOT - 1]
    out = np.empty((B, LQ, D), np.float32)
    for c in range(N_CORES):
        o = np.asarray(res.results[c]["o"], np.float32).reshape(NSLOT, QT, D)
        o2 = np.asarray(res.results[c]["o2"], np.float32)  # [65, 512]
        o[last] = (o2[:D] / o2[D : D + 1]).T
        for s, (b, qt) in enumerate(core_units[c]):
            out[b, qt * QT : (qt + 1) * QT, :] = o[s]
    return out


def kernel(queries, keys, values, valid_lens):
    core_units, nv_slots = _plan(valid_lens)
    nc = _build_module(nv_slots)
    in_maps = _shard_inputs(queries, keys, values, valid_lens, core_units)
    res = bass_utils.run_bass_kernel_spmd(nc, in_maps, core_ids=list(range(N_CORES)))
    return _gather(res, core_units)


# revision 15
# speedup vs baseline: 1.0900x; 1.0220x over previous
"""Fused masked-softmax attention (DotProductAttention) for 8 TRN2 NeuronCores.

Problem: B=16 batches of Q[2048,64] @ K[2048,64]^T -> mask cols >= valid_len
to -1e6 -> softmax -> @ V[2048,64].

Work decomposition: each batch splits into 4 q-quarters of 512 rows (one
PSUM-bank-wide q-tile each) -> 64 independent units.  Units are sorted by
valid k-tile count nv = ceil(valid_len/128) and dealt into 8 SPMD slots of
8 units (one per core); the compiled program runs slot s with a static
nv_s = max over that slot's units.  The mask lives entirely in the V-side
operand (see below), so k-tiles past a unit's valid_len contribute exactly
0 and skipped tiles are exact.  For uniform-random valid_lens this cuts
total work to ~50-60% of dense; worst case (all full) equals dense.

Per-unit kernel (all on-chip, scores never touch HBM):
  * Layout: S^T[k, q] so softmax's k-reduction becomes a matmul and the
    attn @ V contraction needs no transpose of the big matrix.
  * MASKING: entirely in Vaug = [V | ones] with rows k >= valid_len zeroed
    on the host.  Invalid keys then add exactly 0 to both the numerator
    (V side) and the denominator (ones side) -- identical to exp(-1e6)=0
    in the reference.  Scores on invalid keys are real (bounded ~e^5 after
    the 0.125 scale), never masked, and simply multiplied by zero.
  * mm1 contraction is therefore pure K^T.T @ Q^T = 64 rows -> ROW-TILED
    PAIRS: tile 2j loads weights into PE array rows 0-63, tile 2j+1 into
    rows 64-127 (kta is host-packed [128, npairs*128]; Q^T is duplicated
    into SBUF partitions 64-127 so rhs/lhsT base partitions match).  The
    two matmuls of a pair run CONCURRENTLY in the array -> mm1 streams
    ~2 k-tiles per 512 cycles, half the v1 cost.
  * exp: ACT engine (the bottleneck: 1 elem/lane/cycle @ 1.2GHz),
    exp(0.125 * x) straight out of PSUM in merged N<=1536 activations
    (per-instruction overhead ~350 cycles), bf16 out.  The exp table is
    preloaded at t~4.5us by a dummy 1-element activation so the ~2.7us
    ACT_TABLE_LOAD overlaps the input DMAs.
  * mm2:  O^T_aug [65, 512q] = sum_k Vaug[ktile].T @ expS^T[ktile] in
    fp32 PSUM; row 64 accumulates the softmax denominator.
  * Pipelining: matmul groups are emitted with a one-group global
    lookahead (mm1[i+1] before exp[i]/mm2[i]) so mm1 pairs stay adjacent
    in the PE queue across group and slot boundaries and ACT never waits
    at slot boundaries.
  * Head: all input DMAs are issued first on the sync/gpsimd queues
    (Scalar queue stays free for ACT), and a short bf16 warmup matmul
    chain starts the HAM clock-ungate (~3.4us of sustained PE activity)
    during the input-DMA wait.
  * finish: copy PSUM->SBUF (f32), PE-transpose 128-col chunks back to
    [q, d] layout, reciprocal of denominator column, per-partition scale,
    one merged DMA per unit issued from the vector queue.
"""

import functools
import math

import numpy as np
import ml_dtypes

import concourse.bacc as bacc
import concourse.tile as tile
from concourse import mybir
from concourse import bass_utils
from concourse.masks import make_identity

B, LQ, LKV, D = 16, 2048, 2048, 64
N_CORES = 8
KT = 128            # k-tile (partition dim of S^T)
QT = 512            # q-rows per unit (= PSUM bank free dim)
NKT = LKV // KT     # 16
NPAIR = NKT // 2    # 8 pair blocks in kta
NSLOT = (B * LQ) // (N_CORES * QT)  # 8 units per core
GROUP = 3           # max k-tiles per PSUM tile / merged activation
PROC_ORDER = [0, 5, 1, 4, 2, 3, 6, 7]  # slot processing order (tail = smallest)
F32 = mybir.dt.float32
BF16 = mybir.dt.bfloat16
# Schraudolph fast-exp on DVE: bf16_bits(exp(0.125*x)) ~ round(x*SCH_A + SCH_B)
# as int16 (bf16 exponent starts at bit 7; C=7.5 calibrated for min RMS ~1.8%).
# Used for the last tile of 3-wide groups outside the two coldest slots --
# offloads ~16% of exp columns from the ACT bottleneck to the idle DVE at
# ~0.8% added global L2 error (budget 2e-2).
SCH_A = 0.125 * 128.0 / math.log(2.0)
SCH_B = 127.0 * 128.0 - 7.5


def _widths(nv):
    """Split nv k-tiles into activation groups of width <=3, avoiding 1-wide
    groups (measured regression) where possible.  2-wide groups go FIRST so
    each slot's first activation has the shortest possible mm1 prefix."""
    threes, rem = divmod(nv, 3)
    if rem == 0:
        return [3] * threes
    if rem == 2:
        return [2] + [3] * threes
    if threes >= 1:
        return [2, 2] + [3] * (threes - 1)
    return [1]


@functools.lru_cache(maxsize=4)
def _build_module(nv_slots):
    nc = bacc.Bacc(None)
    qta_d = nc.dram_tensor("qta", [128, NSLOT * QT], BF16, kind="ExternalInput")
    kta_d = nc.dram_tensor("kta", [NSLOT, 128, NPAIR * KT], BF16, kind="ExternalInput")
    vau_d = nc.dram_tensor("vaug", [128, NSLOT * NKT * (D + 1)], BF16, kind="ExternalInput")
    out_d = nc.dram_tensor("o", [NSLOT, QT, D], BF16, kind="ExternalOutput")
    # last-processed slot: raw O^T + denominator row, normalized on host
    # (skips the transpose/normalize chain that would serialize at the tail)
    o2_d = nc.dram_tensor("o2", [D + 1, QT], F32, kind="ExternalOutput")

    slot_groups = []
    for nv in nv_slots:
        groups, g = [], 0
        for w in _widths(nv):
            groups.append((g, w))
            g += w
        assert g == nv
        slot_groups.append(groups)

    # Process slots big/small interleaved: small slots are dominated by their
    # finish chains, which hide under the neighboring big slots' long ACT
    # windows.  End on the two smallest slots so the exposed tail is minimal.
    proc_order = PROC_ORDER[:NSLOT]
    # flattened (slot, group_start, width) sequence in emission order
    seq = [(s, g, w) for s in proc_order for (g, w) in slot_groups[s]]

    with tile.TileContext(nc) as tc:
        with (
            tc.tile_pool(name="weights", bufs=1) as wpool,
            tc.tile_pool(name="exps", bufs=3) as epool,
            tc.tile_pool(name="ot", bufs=2) as otpool,
            tc.tile_pool(name="recip", bufs=2) as rpool,
            tc.tile_pool(name="outs", bufs=2) as opool,
            tc.tile_pool(name="ps_s", bufs=2, space="PSUM") as ps_s,
            tc.tile_pool(name="ps_o", bufs=1, space="PSUM") as ps_o,
            tc.tile_pool(name="ps_t", bufs=1, space="PSUM") as ps_t,
        ):
            # --- tiny setup tiles on gpsimd so dependents start early
            adum = wpool.tile([128, 1], F32, tag="adum", name="adum")
            nc.gpsimd.memset(adum, 0.0)
            wk = wpool.tile([64, KT], BF16, tag="wk", name="wk")
            nc.gpsimd.memset(wk, 0.125)
            wq = wpool.tile([64, QT], BF16, tag="wq", name="wq")
            nc.gpsimd.memset(wq, 0.125)

            # --- input SBUF tiles
            qta_all = wpool.tile([128, NSLOT * QT], BF16, tag="qta", name="qta")
            kta_s = [
                wpool.tile(
                    [128, ((nv_slots[s] + 1) // 2) * KT], BF16,
                    tag=f"kta{s}", name=f"kta{s}",
                )
                for s in range(NSLOT)
            ]
            vaug_s = [
                wpool.tile(
                    [128, nv_slots[s] * (D + 1)], BF16,
                    tag=f"vaug{s}", name=f"vaug{s}",
                )
                for s in range(NSLOT)
            ]

            s0 = proc_order[0]
            np0 = (nv_slots[s0] + 1) // 2
            c0 = ((slot_groups[s0][0][1] + 1) // 2) * KT  # first group's pair block(s)
            # slot-0 critical inputs first; qta0 halves go out on two queues
            # in parallel to halve the head-path transfer time
            nc.sync.dma_start(out=qta_all[:64, s0 * QT : (s0 + 1) * QT],
                              in_=qta_d[:64, s0 * QT : (s0 + 1) * QT])
            nc.gpsimd.dma_start(out=qta_all[64:, s0 * QT : (s0 + 1) * QT],
                                in_=qta_d[64:, s0 * QT : (s0 + 1) * QT])
            nc.sync.dma_start(out=kta_s[s0][:, :c0], in_=kta_d[s0, :, :c0])

            # scalar queue: preload the exp ACT table during the DMA wait
            aout = wpool.tile([128, 1], F32, tag="aout", name="aout")
            nc.scalar.activation(out=aout, in_=adum,
                                 func=mybir.ActivationFunctionType.Exp)

            # gpsimd queue: vaug for slot 0, identity, then remaining vaug
            nc.gpsimd.dma_start(
                out=vaug_s[s0],
                in_=vau_d[:, s0 * NKT * (D + 1) : (s0 * NKT + nv_slots[s0]) * (D + 1)],
            )
            ident = wpool.tile([128, 128], F32, tag="ident")
            make_identity(nc, ident)

            # tensor queue: bf16 warmup chain keeps the PE busy through the
            # input-DMA wait (HAM/power-arbiter activity clock)
            warm = ps_s.tile([128, GROUP * QT], F32, tag="st", name="warm")
            for _ in range(8):
                nc.tensor.matmul(warm[:, :QT], lhsT=wk, rhs=wq, start=True, stop=True)

            # rest of the inputs, in consumption order; kta on sync,
            # vaug+qta interleaved on gpsimd
            rest = [s for s in proc_order if s != s0]
            nc.sync.dma_start(
                out=kta_s[s0][:, c0:], in_=kta_d[s0, :, c0 : np0 * KT]
            )
            for s in rest:
                nc.sync.dma_start(
                    out=kta_s[s], in_=kta_d[s, :, : ((nv_slots[s] + 1) // 2) * KT]
                )
            for s in rest:
                nc.gpsimd.dma_start(
                    out=vaug_s[s],
                    in_=vau_d[:, s * NKT * (D + 1) : (s * NKT + nv_slots[s]) * (D + 1)],
                )
                nc.gpsimd.dma_start(out=qta_all[:, s * QT : (s + 1) * QT],
                                    in_=qta_d[:, s * QT : (s + 1) * QT])

            def finish(s, po):
                """Normalize po [65, 512] and store as out[s]."""
                ot = otpool.tile([D + 1, QT], F32, tag="ot", name="ot")
                nc.vector.tensor_copy(ot, po)
                if s == proc_order[-1]:
                    # last slot: ship raw O^T+denom, normalize on host
                    nc.sync.dma_start(out=o2_d[:, :], in_=ot)
                    return
                pt = ps_t.tile([128, QT // 128, D + 1], F32, tag="pt", name="pt")
                for j in range(QT // 128):
                    nc.tensor.transpose(
                        pt[:, j, :],
                        ot[:, j * 128 : (j + 1) * 128],
                        ident[: D + 1, : D + 1],
                    )
                rc = rpool.tile([128, QT // 128], F32, tag="rc", name="rc")
                nc.vector.reciprocal(rc, pt[:, :, D])
                ob = opool.tile([128, QT // 128, D], BF16, tag="ob", name="ob")
                for j in range(QT // 128):
                    nc.vector.tensor_scalar_mul(
                        ob[:, j, :], pt[:, j, :D], rc[:, j : j + 1]
                    )
                out_ap = out_d[s].rearrange("(j p) d -> p j d", p=128)
                nc.sync.dma_start(out=out_ap, in_=ob)

            exps_t, po_t, st_t = {}, {}, {}

            def emit_m1(i):
                s, g, w = seq[i]
                if s not in exps_t:
                    exps_t[s] = epool.tile(
                        [128, nv_slots[s] * QT], BF16, tag="exps", name=f"exps{s}"
                    )
                    po_t[s] = ps_o.tile([D + 1, QT], F32, tag="po", name=f"po{s}")
                st = ps_s.tile([128, GROUP * QT], F32, tag="st", name=f"st{i}")
                st_t[i] = st
                for j in range(w):
                    n = g + j
                    h, jp = n % 2, n // 2
                    nc.tensor.matmul(
                        st[:, j * QT : (j + 1) * QT],
                        lhsT=kta_s[s][h * 64 : (h + 1) * 64, jp * KT : (jp + 1) * KT],
                        rhs=qta_all[h * 64 : (h + 1) * 64, s * QT : (s + 1) * QT],
                        start=True,
                        stop=True,
                    )

            proc_idx = {s: k for k, s in enumerate(proc_order)}
            emit_m1(0)
            for i, (s, g, w) in enumerate(seq):
                if i + 1 < len(seq):
                    emit_m1(i + 1)
                st = st_t.pop(i)
                nv = nv_slots[s]
                # last tile of 3-wide groups -> DVE fast-exp, except in the
                # first two processed slots (PE is power-throttled there and
                # can't afford the faster ACT cadence anyway)
                use_dve = w == 3 and proc_idx[s] >= 2
                wa = 2 if use_dve else w
                nc.scalar.activation(
                    out=exps_t[s][:, g * QT : (g + wa) * QT],
                    in_=st[:, : wa * QT],
                    func=mybir.ActivationFunctionType.Exp,
                    scale=0.125,
                )
                if use_dve:
                    n = g + 2
                    nc.vector.tensor_scalar(
                        out=exps_t[s][:, n * QT : (n + 1) * QT].bitcast(
                            mybir.dt.int16
                        ),
                        in0=st[:, 2 * QT : 3 * QT],
                        scalar1=SCH_A,
                        scalar2=SCH_B,
                        op0=mybir.AluOpType.mult,
                        op1=mybir.AluOpType.add,
                    )
                for j in range(w):
                    n = g + j
                    nc.tensor.matmul(
                        po_t[s],
                        lhsT=vaug_s[s][:, n * (D + 1) : (n + 1) * (D + 1)],
                        rhs=exps_t[s][:, n * QT : (n + 1) * QT],
                        start=(n == 0),
                        stop=(n == nv - 1),
                        skip_group_check=True,
                    )
                if g + w == nv:
                    finish(s, po_t[s])

    nc.compile()
    return nc


def _plan(valid_lens):
    """Sort the 64 (batch, q-quarter) units by valid k-tile count and deal
    them into NSLOT slots of one unit per core.  Returns (core_units,
    nv_slots) where core_units[c][s] = (batch, quarter)."""
    VL = np.asarray(valid_lens).astype(np.int64)
    nv = np.maximum(1, np.minimum(NKT, (VL + KT - 1) // KT))
    qpb = LQ // QT  # quarters per batch
    unit_nv = np.repeat(nv, qpb)
    order = np.argsort(-unit_nv, kind="stable")
    core_units = [
        [(int(order[NSLOT * s + c]) // qpb, int(order[NSLOT * s + c]) % qpb) for s in range(NSLOT)]
        for c in range(N_CORES)
    ]
    nv_slots = tuple(int(unit_nv[order[NSLOT * s]]) for s in range(NSLOT))
    return core_units, nv_slots


def _shard_inputs(queries, keys, values, valid_lens, core_units):
    """Host-side layout per core: stacked per-unit operands.

    kta: pure K^T pair-packed [128, NPAIR*128]: partition (h*64+d), col
         (j*128+c) = K[b][(2j+h)*128+c, d] -- tile 2j in partitions 0-63,
         tile 2j+1 in partitions 64-127 (PE row-tiling halves).
    qta: Q^T quarter duplicated into partitions 64-127: [128, 512].
    vaug: [V | ones] with rows k >= valid_len zeroed (the mask lives here).
    """
    Q = np.asarray(queries, dtype=np.float32)
    K = np.asarray(keys, dtype=np.float32)
    V = np.asarray(values, dtype=np.float32)
    VL = np.asarray(valid_lens).astype(np.int64)

    cols = np.arange(LKV, dtype=np.int64)
    in_maps = []
    for c in range(N_CORES):
        qta = np.empty((128, NSLOT * QT), np.float32)
        kta = np.empty((NSLOT, 128, NPAIR * KT), np.float32)
        va = np.empty((128, NSLOT * NKT * (D + 1)), np.float32)
        for s, (b, qt) in enumerate(core_units[c]):
            qT = Q[b, qt * QT : (qt + 1) * QT, :].T  # [64, 512]
            qta[:64, s * QT : (s + 1) * QT] = qT
            qta[64:, s * QT : (s + 1) * QT] = qT
            # K[b] [2048, 64] -> [NPAIR, 2, KT, D] -> (h, d, j, c)
            kta[s] = (
                K[b].reshape(NPAIR, 2, KT, D).transpose(1, 3, 0, 2).reshape(128, -1)
            )
            vb = np.concatenate([V[b], np.ones((LKV, 1), np.float32)], axis=-1)
            vb *= (cols < VL[b])[:, None]
            va[:, s * NKT * (D + 1) : (s + 1) * NKT * (D + 1)] = (
                vb.reshape(NKT, KT, D + 1).transpose(1, 0, 2).reshape(128, -1)
            )
        in_maps.append(
            {
                "qta": qta.astype(ml_dtypes.bfloat16),
                "kta": kta.astype(ml_dtypes.bfloat16),
                "vaug": va.astype(ml_dtypes.bfloat16),
            }
        )
    return in_maps


def _gather(res, core_units):
    """Assemble the full [B, LQ, D] f32 output from per-core results."""
    last = PROC_ORDER[NSLOT - 1]
    out = np.empty((B, LQ, D), np.float32)
    for c in range(N_CORES):
        o = np.asarray(res.results[c]["o"], np.float32).reshape(NSLOT, QT, D)
        o2 = np.asarray(res.results[c]["o2"], np.float32)  # [65, 512]
        o[last] = (o2[:D] / o2[D : D + 1]).T
        for s, (b, qt) in enumerate(core_units[c]):
            out[b, qt * QT : (qt + 1) * QT, :] = o[s]
    return out


def kernel(queries, keys, values, valid_lens):
    core_units, nv_slots = _plan(valid_lens)
    nc = _build_module(nv_slots)
    in_maps = _shard_inputs(queries, keys, values, valid_lens, core_units)
    res = bass_utils.run_bass_kernel_spmd(nc, in_maps, core_ids=list(range(N_CORES)))
    return _gather(res, core_units)
